# revision 1
# baseline (speedup 1.0000x reference)
"""Blockwise butterfly rotation (nn_BlockwiseButterflyRotation) - TRN2 Bass kernel.

Full inputs: x (4, 4096, 4096) f32, angles (16, 8, 128) f32.
Math: x is split into 16 independent 256-wide blocks; each block's rows are
rotated by an 8-stage butterfly. The composed per-block rotation is a dense
256x256 matrix C_b = B_b^T, so out = x @ blockdiag(C). The kernel builds C
on-device from the angles and runs the bulk work as PE matmuls.

Sharding: data-parallel over rows - x.reshape(16384, 4096) split into 8
contiguous shards of 2048 rows; angles (gathered into per-partition coeff
layout, pure indexing) replicated to all cores.

Per-core dataflow, per 128-row tile:
  DMA in [128, 4096] f32
  -> 32x PE transpose (f32, exact) of 128x128 chunks -> PSUM
  -> PSUM->SBUF copy, rounding to float32r (DVE/ACT alternating)
  -> 32x PE matmul f32r: out[128, 256] += xT_chunk^T @ C_chunk (N=256 ->
     full PE rate for f32r)
  -> PSUM->SBUF copy f32 -> DMA out

C build (once, overlapped with the first row tiles): two-level butterfly
factorization C[16g+u, 16w+v] = LT_g[u,v] * HT_v[g,w]; LT (stages 0-3) and
HT (stages 4-7) are built by applying 16x16 butterflies to identity patterns
with free-dim-only pairing on the DVE; cos/sin via ScalarE Sin (cos =
sin(x + pi/2)); HT's u-replication via 16 selector matmuls on the PE,
streamed per block; the combine writes float32r directly. Constant 0/1
init patterns (identity, butterfly-init deltas, selector matrices) are
shipped as one small constant input so no long GPSIMD init chain sits on
the critical path, and the build's SBUF footprint is kept small enough
that it coexists with the main-loop pools (build fully overlaps the
first input DMAs).
"""
import math
import os

import numpy as np

from concourse import bacc, mybir, tile
from concourse.bass_utils import run_bass_kernel_spmd

F32 = mybir.dt.float32
F32R = mybir.dt.float32r

DIM = 4096
NB = 16
BLOCK = 256
HALF_PI = math.pi / 2.0

N_CORES = 8
R_TOTAL = 4 * 4096
R_CORE = R_TOTAL // N_CORES  # 2048

# consts tensor column layout: ident | halfpi | LSinit | HSBinit | W_all
_C_ID = 0          # [128, 128] identity (PE transpose operand)
_C_PI = 128        # [128, 1] pi/2
_C_LS = 129        # [128, 512] LS init: delta(v == p mod 16), free (b, kc, v)
_C_HSB = 641       # [128, 512] HSB init: delta(w == 8kc + p//16), free (kc, v, w)
_C_W = 1153        # [128, 2048] W_all: free (b, mg, mu), delta(p == 16 mg + b)
_C_COLS = 3201

LAST_RESULT = None  # BassKernelResults of the most recent kernel() call
_NC_CACHE = {}


def _build_consts() -> np.ndarray:
    c = np.zeros((128, _C_COLS), dtype=np.float32)
    p = np.arange(128)
    c[:, _C_ID:_C_ID + 128] = np.eye(128, dtype=np.float32)
    c[:, _C_PI] = HALF_PI
    ls = np.zeros((128, 16, 2, 16), np.float32)
    ls[p, :, :, p % 16] = 1.0
    c[:, _C_LS:_C_LS + 512] = ls.reshape(128, 512)
    hsb = np.zeros((128, 2, 16, 16), np.float32)
    for kc in range(2):
        hsb[:, kc, :, :] = (np.arange(16)[None, :] == (8 * kc + p // 16)[:, None])[:, None, :]
    c[:, _C_HSB:_C_HSB + 512] = hsb.reshape(128, 512)
    w = np.zeros((128, 16, 8, 16), np.float32)
    for b in range(16):
        for mg in range(8):
            w[16 * mg + b, b, mg, :] = 1.0
    c[:, _C_W:_C_W + 2048] = w.reshape(128, 2048)
    return c


_CONSTS = _build_consts()


def gather_angles(angles: np.ndarray) -> np.ndarray:
    """angles [16, 8, 128] f32 -> ang [128, 1536] f32 (angL 4x256 | angH 4x128).

    Pure gather (indexing only, no arithmetic) into the per-partition
    coefficient layouts the kernel's butterfly-stage APs iterate.
    """
    angles = np.asarray(angles)
    assert angles.shape == (NB, 8, 128)
    out = np.empty((128, 1536), dtype=np.float32)
    for s in range(4):
        sig = 1 << s
        col = np.empty((128, 256), dtype=np.float32)
        for g0 in range(8):
            row = np.empty((16, 2, 8), dtype=np.float32)
            for kc in range(2):
                g = 8 * kc + g0
                for vg in range(8 // sig):
                    for t in range(sig):
                        row[:, kc, vg * sig + t] = angles[:, s, 8 * g + vg * sig + t]
            col[16 * g0:16 * g0 + 16, :] = row.reshape(1, 256)
        out[:, 256 * s:256 * (s + 1)] = col
    for sp in range(4):
        sigp = 1 << sp
        col = np.empty((128, 128), dtype=np.float32)
        for b in range(16):
            row = np.empty((16, 8), dtype=np.float32)
            for v in range(16):
                for wg in range(8 // sigp):
                    for t in range(sigp):
                        row[v, wg * sigp + t] = angles[b, sp + 4, wg * 16 * sigp + 16 * t + v]
            col[b::16, :] = row.reshape(1, 128)
        out[:, 1024 + 128 * sp:1024 + 128 * (sp + 1)] = col
    return out


def _butterfly_stage(nc, pool, data, n1, n2, sig, cos_ap, sin_ap):
    """One butterfly stage on `data` viewed as [p, n1, n2, ng, 2, sig];
    pairs along the (ng, 2, sig) axis group. cos/sin APs iterate
    [p, n1, n2, ng, sig]."""
    ng = 8 // sig
    v = data.rearrange("p (n1 n2 vg h t) -> p n1 n2 vg h t",
                       n1=n1, n2=n2, vg=ng, h=2, t=sig)
    a = v[:, :, :, :, 0, :]
    b_ = v[:, :, :, :, 1, :]
    half = n1 * n2 * 8
    t1 = pool.tile([128, half], F32, name="bt_t1", tag="bt_t1")
    t2 = pool.tile([128, half], F32, name="bt_t2", tag="bt_t2")
    t3 = pool.tile([128, half], F32, name="bt_t3", tag="bt_t3")
    t4 = pool.tile([128, half], F32, name="bt_t4", tag="bt_t4")
    tv = lambda t: t[:].rearrange("p (n1 n2 vg t) -> p n1 n2 vg t",
                                  n1=n1, n2=n2, vg=ng, t=sig)
    nc.vector.tensor_mul(tv(t1), a, cos_ap)
    nc.vector.tensor_mul(tv(t2), b_, sin_ap)
    nc.vector.tensor_mul(tv(t3), a, sin_ap)
    nc.vector.tensor_mul(tv(t4), b_, cos_ap)
    nc.vector.tensor_sub(a, tv(t1), tv(t2))
    nc.vector.tensor_add(b_, tv(t3), tv(t4))


def build_nc(R: int, repeat: int | None = None, repeat_scope: str = "main"):
    """repeat: if set, wrap the kernel body in an on-device For_i that re-runs
    it `repeat` times on the same data (identical output; used by the timing
    harness to resolve per-pass time above the dispatch noise floor).
    repeat_scope: "main" loops only the row-tile loop; "all" also loops the
    C build."""
    import contextlib
    assert R % 128 == 0
    RT = R // 128
    nc = bacc.Bacc("TRN2", target_bir_lowering=False, debug=False)

    X = nc.dram_tensor("x", [R, DIM], F32, kind="ExternalInput").ap()
    ANG = nc.dram_tensor("ang", [128, 1536], F32, kind="ExternalInput").ap()
    CIN = nc.dram_tensor("consts", [128, _C_COLS], F32, kind="ExternalInput").ap()
    OUT = nc.dram_tensor("out", [R, DIM], F32, kind="ExternalOutput").ap()

    with tile.TileContext(nc) as tc:
        with tc.tile_pool(name="const", bufs=1) as cpool, \
             tc.tile_pool(name="build", bufs=1) as bpool, \
             tc.tile_pool(name="xin", bufs=3) as xpool, \
             tc.tile_pool(name="xt", bufs=2) as xtpool, \
             tc.tile_pool(name="outp", bufs=2) as opool, \
             tc.tile_pool(name="psR", bufs=2, space="PSUM") as psR, \
             tc.tile_pool(name="psT", bufs=3, space="PSUM") as psT, \
             tc.tile_pool(name="psO", bufs=3, space="PSUM") as psO:
            consts = cpool.tile([128, _C_COLS], F32)
            nc.sync.dma_start(out=consts[:], in_=CIN)
            ident = consts[:, _C_ID:_C_ID + 128]
            halfpi = consts[:, _C_PI:_C_PI + 1]
            CT = cpool.tile([128, 8192], F32R)  # C: [p=k%128, (b, kc, w, v)]

            outer_cm = (tc.For_i(0, repeat, 1)
                        if repeat and repeat_scope == "all"
                        else contextlib.nullcontext())
            outer_cm.__enter__()

            # ---------------- C build ----------------
            angsb = bpool.tile([128, 1536], F32)
            nc.sync.dma_start(out=angsb[:], in_=ANG)

            # LS [p=(g0,u), (b:16, kc:2, v:16)], HSB [p=(g0,b), (kc:2, v:16, w:16)]
            LS = bpool.tile([128, 512], F32)
            nc.vector.tensor_copy(LS[:], consts[:, _C_LS:_C_LS + 512])
            HSB = bpool.tile([128, 512], F32)
            nc.vector.tensor_copy(HSB[:], consts[:, _C_HSB:_C_HSB + 512])

            # HSB stages 4-7 first (they gate the longest downstream chain);
            # coeffs independent of kc -> broadcast over kc.
            for sp in range(4):
                sigp = 1 << sp
                ng = 8 // sigp
                cosT = bpool.tile([128, 128], F32, name="cosH", tag="cosH")
                sinT = bpool.tile([128, 128], F32, name="sinH", tag="sinH")
                asl = angsb[:, 1024 + 128 * sp:1024 + 128 * (sp + 1)]
                nc.scalar.activation(cosT[:], asl, mybir.ActivationFunctionType.Sin,
                                     bias=halfpi, scale=1.0)
                nc.scalar.activation(sinT[:], asl, mybir.ActivationFunctionType.Sin,
                                     bias=0.0, scale=1.0)
                cv = cosT[:].rearrange("p (v vg t) -> p v vg t", v=16, vg=ng, t=sigp) \
                    .unsqueeze(1).to_broadcast((128, 2, 16, ng, sigp))
                sv = sinT[:].rearrange("p (v vg t) -> p v vg t", v=16, vg=ng, t=sigp) \
                    .unsqueeze(1).to_broadcast((128, 2, 16, ng, sigp))
                _butterfly_stage(nc, bpool, HSB[:], 2, 16, sigp, cv, sv)

            # LS stages 0-3
            for s in range(4):
                sig = 1 << s
                ng = 8 // sig
                cosT = bpool.tile([128, 256], F32, name="cosL", tag="cosL")
                sinT = bpool.tile([128, 256], F32, name="sinL", tag="sinL")
                asl = angsb[:, 256 * s:256 * (s + 1)]
                nc.scalar.activation(cosT[:], asl, mybir.ActivationFunctionType.Sin,
                                     bias=halfpi, scale=1.0)
                nc.scalar.activation(sinT[:], asl, mybir.ActivationFunctionType.Sin,
                                     bias=0.0, scale=1.0)
                cv = cosT[:].rearrange("p (b kc vg t) -> p b kc vg t",
                                       b=16, kc=2, vg=ng, t=sig)
                sv = sinT[:].rearrange("p (b kc vg t) -> p b kc vg t",
                                       b=16, kc=2, vg=ng, t=sig)
                _butterfly_stage(nc, bpool, LS[:], 16, 2, sig, cv, sv)

            # Per block b: replicate HSB group-row b to all u-lanes via the PE
            # (HS_b[16g0+u, (kc,v,w)] = HSB[16g0+b, (kc,v,w)]), then combine:
            # CT[p, b, kc, w, v] = LS[p, b, kc, v] * HS_b[p, kc, v, w] (f32r out).
            for b in range(16):
                Wb = consts[:, _C_W + 128 * b:_C_W + 128 * (b + 1)]
                psr = psR.tile([128, 512], F32, name="psr", tag="psr")
                nc.tensor.matmul(psr[:], Wb, HSB[:], start=True, stop=True)
                hss = bpool.tile([128, 512], F32, name="hss", tag="hss", bufs=2)
                if b % 2 == 0:
                    nc.vector.tensor_copy(hss[:], psr[:])
                else:
                    nc.scalar.copy(hss[:], psr[:])
                for kc in range(2):
                    o = CT[:, (b * 2 + kc) * 256:(b * 2 + kc) * 256 + 256] \
                        .rearrange("p (w v) -> p w v", w=16, v=16)
                    i0 = LS[:, (b * 32 + kc * 16):(b * 32 + kc * 16) + 16] \
                        .unsqueeze(1).to_broadcast((128, 16, 16))
                    i1 = hss[:, 256 * kc:256 * (kc + 1)] \
                        .rearrange("p (v w) -> p w v", v=16, w=16)
                    nc.vector.tensor_mul(o, i0, i1)

            # ---------------- main loop ----------------
            inner_cm = (tc.For_i(0, repeat, 1)
                        if repeat and repeat_scope == "main"
                        else contextlib.nullcontext())
            with inner_cm:
                for r in range(RT):
                    xin = xpool.tile([128, DIM], F32, name="xin", tag="xin")
                    nc.sync.dma_start(out=xin[:], in_=X[r * 128:(r + 1) * 128, :])

                    xT = xtpool.tile([128, DIM], F32R, name="xT", tag="xT")
                    for j in range(8):
                        pst = psT.tile([128, 512], F32, name="pst", tag="pst")
                        for q in range(4):
                            i = 4 * j + q
                            nc.tensor.transpose(
                                pst[:, 128 * q:128 * (q + 1)],
                                xin[:, 128 * i:128 * (i + 1)], ident)
                        if j % 2 == 0:
                            nc.vector.tensor_copy(xT[:, 512 * j:512 * (j + 1)], pst[:])
                        else:
                            nc.scalar.copy(xT[:, 512 * j:512 * (j + 1)], pst[:])

                    outt = opool.tile([128, DIM], F32, name="outt", tag="outt")
                    for jb in range(8):
                        pso = psO.tile([128, 512], F32, name="pso", tag="pso")
                        for q in range(2):
                            b = 2 * jb + q
                            for kc in range(2):
                                i = 2 * b + kc
                                nc.tensor.matmul(
                                    pso[:, 256 * q:256 * (q + 1)],
                                    xT[:, 128 * i:128 * (i + 1)],
                                    CT[:, 256 * i:256 * (i + 1)],
                                    start=(kc == 0), stop=(kc == 1))
                        if jb % 2 == 0:
                            nc.vector.tensor_copy(outt[:, 512 * jb:512 * (jb + 1)], pso[:])
                        else:
                            nc.scalar.copy(outt[:, 512 * jb:512 * (jb + 1)], pso[:])
                    nc.gpsimd.dma_start(out=OUT[r * 128:(r + 1) * 128, :], in_=outt[:])
            outer_cm.__exit__(None, None, None)

    nc.compile()
    return nc


def _get_nc():
    if "nc" not in _NC_CACHE:
        _NC_CACHE["nc"] = build_nc(R_CORE)
    return _NC_CACHE["nc"]


def kernel(x: np.ndarray, angles: np.ndarray) -> np.ndarray:
    global LAST_RESULT
    x = np.asarray(x)
    angles = np.asarray(angles)
    orig_shape = x.shape
    xf = np.ascontiguousarray(x.reshape(R_TOTAL, DIM), dtype=np.float32)
    ang = gather_angles(angles.astype(np.float32, copy=False))

    nc = _get_nc()
    in_maps = [
        {"x": np.ascontiguousarray(xf[c * R_CORE:(c + 1) * R_CORE]),
         "ang": ang, "consts": _CONSTS}
        for c in range(N_CORES)
    ]
    trace = os.environ.get("BFK_TRACE", "") == "1"
    res = run_bass_kernel_spmd(nc, in_maps, list(range(N_CORES)), trace=trace)
    LAST_RESULT = res
    out = np.concatenate([res.results[c]["out"] for c in range(N_CORES)], axis=0)
    return out.reshape(orig_shape).astype(x.dtype, copy=False)



# revision 2
# speedup vs baseline: 2.4307x; 2.4307x over previous
"""Blockwise butterfly rotation - TRN2 Bass kernel, v5.

out = x @ blockdiag(C_b); C built on device from angles; PE does 32 bf16
transposes + 32 bf16 matmuls per 128-row tile (measured ~6.1us/tile incl.
PSUM drains); DMA: f32 in (HWDGE sync), bf16 out (SWDGE, no cast).

v5 vs v4:
  - C build slimmed: butterfly stages + combine in bf16 (DVE chain ~20K
    cycles vs ~40K), hss PSUM->SBUF copies moved to ACT (idle during build),
    combine merged to one 512-wide mul per (block, kc-pair) -> fewer
    per-op overheads.
  - Engine rebalance for the steady state: DVE = x conversion + 4 psT + 6
    psO drains (~4.5us/tile), ACT = 4 psT + 2 psO (~3.4us/tile), both under
    the PE's ~6.1us/tile.
  - xT buffered 8 deep + psT 6 deep so the PE's transpose pipeline covers
    the ~20us CT-build window without stalling.
"""
import math
import os

import numpy as np

from concourse import bacc, mybir, tile
from concourse.bass_utils import run_bass_kernel_spmd

F32 = mybir.dt.float32
BF16 = mybir.dt.bfloat16

DIM = 4096
NB = 16
BLOCK = 256
HALF_PI = math.pi / 2.0

N_CORES = 8
R_TOTAL = 4 * 4096
R_CORE = R_TOTAL // N_CORES  # 2048

_C_ID = 0
_C_PI = 128
_C_LS = 129
_C_HSB = 641
_C_W = 1153
_C_COLS = 3201

LAST_RESULT = None
_NC_CACHE = {}

BUILD_DT = mybir.dt.bfloat16 if os.environ.get("BFK_BUILD", "bf16") == "bf16" \
    else mybir.dt.float32


def _build_consts() -> np.ndarray:
    c = np.zeros((128, _C_COLS), dtype=np.float32)
    p = np.arange(128)
    c[:, _C_ID:_C_ID + 128] = np.eye(128, dtype=np.float32)
    c[:, _C_PI] = HALF_PI
    ls = np.zeros((128, 16, 2, 16), np.float32)
    ls[p, :, :, p % 16] = 1.0
    c[:, _C_LS:_C_LS + 512] = ls.reshape(128, 512)
    hsb = np.zeros((128, 2, 16, 16), np.float32)
    for kc in range(2):
        hsb[:, kc, :, :] = (np.arange(16)[None, :] == (8 * kc + p // 16)[:, None])[:, None, :]
    c[:, _C_HSB:_C_HSB + 512] = hsb.reshape(128, 512)
    w = np.zeros((128, 16, 8, 16), np.float32)
    for b in range(16):
        for mg in range(8):
            w[16 * mg + b, b, mg, :] = 1.0
    c[:, _C_W:_C_W + 2048] = w.reshape(128, 2048)
    return c


_CONSTS = _build_consts()


def gather_angles(angles: np.ndarray) -> np.ndarray:
    """angles [16, 8, 128] f32 -> ang [128, 1536] f32 (angL 4x256 | angH 4x128)."""
    angles = np.asarray(angles)
    assert angles.shape == (NB, 8, 128)
    out = np.empty((128, 1536), dtype=np.float32)
    for s in range(4):
        sig = 1 << s
        col = np.empty((128, 256), dtype=np.float32)
        for g0 in range(8):
            row = np.empty((16, 2, 8), dtype=np.float32)
            for kc in range(2):
                g = 8 * kc + g0
                for vg in range(8 // sig):
                    for t in range(sig):
                        row[:, kc, vg * sig + t] = angles[:, s, 8 * g + vg * sig + t]
            col[16 * g0:16 * g0 + 16, :] = row.reshape(1, 256)
        out[:, 256 * s:256 * (s + 1)] = col
    for sp in range(4):
        sigp = 1 << sp
        col = np.empty((128, 128), dtype=np.float32)
        for b in range(16):
            row = np.empty((16, 8), dtype=np.float32)
            for v in range(16):
                for wg in range(8 // sigp):
                    for t in range(sigp):
                        row[v, wg * sigp + t] = angles[b, sp + 4, wg * 16 * sigp + 16 * t + v]
            col[b::16, :] = row.reshape(1, 128)
        out[:, 1024 + 128 * sp:1024 + 128 * (sp + 1)] = col
    return out


def _butterfly_stage(nc, pool, data, n1, n2, sig, cos_ap, sin_ap, dt):
    ng = 8 // sig
    v = data.rearrange("p (n1 n2 vg h t) -> p n1 n2 vg h t",
                       n1=n1, n2=n2, vg=ng, h=2, t=sig)
    a = v[:, :, :, :, 0, :]
    b_ = v[:, :, :, :, 1, :]
    half = n1 * n2 * 8
    t1 = pool.tile([128, half], dt, name="bt_t1", tag="bt_t1")
    t2 = pool.tile([128, half], dt, name="bt_t2", tag="bt_t2")
    t3 = pool.tile([128, half], dt, name="bt_t3", tag="bt_t3")
    t4 = pool.tile([128, half], dt, name="bt_t4", tag="bt_t4")
    tv = lambda t: t[:].rearrange("p (n1 n2 vg t) -> p n1 n2 vg t",
                                  n1=n1, n2=n2, vg=ng, t=sig)
    nc.vector.tensor_mul(tv(t1), a, cos_ap)
    nc.vector.tensor_mul(tv(t2), b_, sin_ap)
    nc.vector.tensor_mul(tv(t3), a, sin_ap)
    nc.vector.tensor_mul(tv(t4), b_, cos_ap)
    nc.vector.tensor_sub(a, tv(t1), tv(t2))
    nc.vector.tensor_add(b_, tv(t3), tv(t4))


def build_nc(R: int, repeat: int | None = None, repeat_scope: str = "main"):
    import contextlib
    assert R % 128 == 0
    RT = R // 128
    nc = bacc.Bacc("TRN2", target_bir_lowering=False, debug=False)
    BDT = BUILD_DT

    X = nc.dram_tensor("x", [R, DIM], F32, kind="ExternalInput").ap()
    ANG = nc.dram_tensor("ang", [128, 1536], F32, kind="ExternalInput").ap()
    CIN = nc.dram_tensor("consts", [128, _C_COLS], F32, kind="ExternalInput").ap()
    OUT = nc.dram_tensor("out", [R, DIM], BF16, kind="ExternalOutput").ap()

    with tile.TileContext(nc) as tc:
        with tc.tile_pool(name="const", bufs=1) as cpool, \
             tc.tile_pool(name="build", bufs=1) as bpool, \
             tc.tile_pool(name="xin", bufs=2) as xpool, \
             tc.tile_pool(name="xinb", bufs=3) as xbpool, \
             tc.tile_pool(name="xt", bufs=8) as xtpool, \
             tc.tile_pool(name="outp", bufs=2) as opool, \
             tc.tile_pool(name="psR", bufs=2, space="PSUM") as psR, \
             tc.tile_pool(name="psT", bufs=3, space="PSUM") as psT, \
             tc.tile_pool(name="psO", bufs=3, space="PSUM") as psO:
            consts = cpool.tile([128, _C_COLS], F32)
            nc.sync.dma_start(out=consts[:], in_=CIN)
            ident = consts[:, _C_ID:_C_ID + 128]
            halfpi = consts[:, _C_PI:_C_PI + 1]
            identb = cpool.tile([128, 128], BF16)
            nc.vector.tensor_copy(identb[:], ident)
            CT = cpool.tile([128, 8192], BF16)  # C: [p=k%128, (b, kc, w, v)]

            outer_cm = (tc.For_i(0, repeat, 1)
                        if repeat and repeat_scope == "all"
                        else contextlib.nullcontext())
            outer_cm.__enter__()

            # ---------------- C build ----------------
            angsb = bpool.tile([128, 1536], F32)
            nc.sync.dma_start(out=angsb[:], in_=ANG)

            LS = bpool.tile([128, 512], BDT)
            nc.vector.tensor_copy(LS[:], consts[:, _C_LS:_C_LS + 512])
            HSB = bpool.tile([128, 512], BDT)
            nc.vector.tensor_copy(HSB[:], consts[:, _C_HSB:_C_HSB + 512])

            # HSB stages 4-7 first (gate the selector matmuls).
            for sp in range(4):
                sigp = 1 << sp
                ng = 8 // sigp
                cosT = bpool.tile([128, 128], BDT, name="cosH", tag="cosH")
                sinT = bpool.tile([128, 128], BDT, name="sinH", tag="sinH")
                asl = angsb[:, 1024 + 128 * sp:1024 + 128 * (sp + 1)]
                nc.scalar.activation(cosT[:], asl, mybir.ActivationFunctionType.Sin,
                                     bias=halfpi, scale=1.0)
                nc.scalar.activation(sinT[:], asl, mybir.ActivationFunctionType.Sin,
                                     bias=0.0, scale=1.0)
                cv = cosT[:].rearrange("p (v vg t) -> p v vg t", v=16, vg=ng, t=sigp) \
                    .unsqueeze(1).to_broadcast((128, 2, 16, ng, sigp))
                sv = sinT[:].rearrange("p (v vg t) -> p v vg t", v=16, vg=ng, t=sigp) \
                    .unsqueeze(1).to_broadcast((128, 2, 16, ng, sigp))
                _butterfly_stage(nc, bpool, HSB[:], 2, 16, sigp, cv, sv, BDT)

            # LS stages 0-3
            for s in range(4):
                sig = 1 << s
                ng = 8 // sig
                cosT = bpool.tile([128, 256], BDT, name="cosL", tag="cosL")
                sinT = bpool.tile([128, 256], BDT, name="sinL", tag="sinL")
                asl = angsb[:, 256 * s:256 * (s + 1)]
                nc.scalar.activation(cosT[:], asl, mybir.ActivationFunctionType.Sin,
                                     bias=halfpi, scale=1.0)
                nc.scalar.activation(sinT[:], asl, mybir.ActivationFunctionType.Sin,
                                     bias=0.0, scale=1.0)
                cv = cosT[:].rearrange("p (b kc vg t) -> p b kc vg t",
                                       b=16, kc=2, vg=ng, t=sig)
                sv = sinT[:].rearrange("p (b kc vg t) -> p b kc vg t",
                                       b=16, kc=2, vg=ng, t=sig)
                _butterfly_stage(nc, bpool, LS[:], 16, 2, sig, cv, sv, BDT)

            # Selector matmuls need HSB in a PE dtype. HSB is bf16 (or f32
            # fallback -> use f32r for full-rate); Wb is f32 -> make a bf16 W.
            Wball = cpool.tile([128, 2048], BF16)
            nc.scalar.copy(Wball[:], consts[:, _C_W:_C_W + 2048])
            HSBmm = HSB
            if BDT != BF16:
                HSBmm = bpool.tile([128, 512], BF16)
                nc.vector.tensor_copy(HSBmm[:], HSB[:])

            for b in range(16):
                Wb = Wball[:, 128 * b:128 * (b + 1)]
                psr = psR.tile([128, 512], F32, name="psr", tag="psr")
                nc.tensor.matmul(psr[:], Wb, HSBmm[:], start=True, stop=True)
                hss = bpool.tile([128, 512], BDT, name="hss", tag="hss", bufs=2)
                nc.scalar.copy(hss[:], psr[:])
                # CT[p, b, kc, w, v] = LS[p, b, kc, v] * hss[p, kc, v, w],
                # one 512-wide mul per block.
                o = CT[:, b * 512:(b + 1) * 512] \
                    .rearrange("p (kc w v) -> p kc w v", kc=2, w=16, v=16)
                i0 = LS[:, b * 32:(b + 1) * 32] \
                    .rearrange("p (kc v) -> p kc v", kc=2, v=16) \
                    .unsqueeze(2).to_broadcast((128, 2, 16, 16))
                i1 = hss[:].rearrange("p (kc v w) -> p kc w v", kc=2, v=16, w=16)
                nc.vector.tensor_mul(o, i0, i1)

            # ---------------- main loop ----------------
            inner_cm = (tc.For_i(0, repeat, 1)
                        if repeat and repeat_scope == "main"
                        else contextlib.nullcontext())
            with inner_cm:
                for r in range(RT):
                    xin32 = xpool.tile([128, DIM], F32, name="xin32", tag="xin32")
                    nc.sync.dma_start(out=xin32[:], in_=X[r * 128:(r + 1) * 128, :])
                    xin = xbpool.tile([128, DIM], BF16, name="xin", tag="xin")
                    nc.vector.tensor_copy(xin[:, 0:3072], xin32[:, 0:3072])
                    nc.scalar.copy(xin[:, 3072:4096], xin32[:, 3072:4096])

                    xT = xtpool.tile([128, DIM], BF16, name="xT", tag="xT")
                    for j in range(8):
                        pst = psT.tile([128, 512], BF16, name="pst", tag="pst")
                        for q in range(4):
                            i = 4 * j + q
                            nc.tensor.transpose(
                                pst[:, 128 * q:128 * (q + 1)],
                                xin[:, 128 * i:128 * (i + 1)], identb[:])
                        if j % 2 == 0:
                            nc.vector.tensor_copy(xT[:, 512 * j:512 * (j + 1)], pst[:])
                        else:
                            nc.scalar.copy(xT[:, 512 * j:512 * (j + 1)], pst[:])

                    outt = opool.tile([128, DIM], BF16, name="outt", tag="outt")
                    for jb in range(8):
                        pso = psO.tile([128, 512], F32, name="pso", tag="pso")
                        for q in range(2):
                            b = 2 * jb + q
                            for kc in range(2):
                                i = 2 * b + kc
                                nc.tensor.matmul(
                                    pso[:, 256 * q:256 * (q + 1)],
                                    xT[:, 128 * i:128 * (i + 1)],
                                    CT[:, 256 * i:256 * (i + 1)],
                                    start=(kc == 0), stop=(kc == 1))
                        if jb in (1, 5):
                            nc.scalar.copy(outt[:, 512 * jb:512 * (jb + 1)], pso[:])
                        else:
                            nc.vector.tensor_copy(outt[:, 512 * jb:512 * (jb + 1)], pso[:])
                    nc.gpsimd.dma_start(out=OUT[r * 128:(r + 1) * 128, :],
                                            in_=outt[:])
            outer_cm.__exit__(None, None, None)

    nc.compile()
    return nc


def _get_nc():
    if "nc" not in _NC_CACHE:
        _NC_CACHE["nc"] = build_nc(R_CORE)
    return _NC_CACHE["nc"]


def kernel(x: np.ndarray, angles: np.ndarray) -> np.ndarray:
    global LAST_RESULT
    x = np.asarray(x)
    angles = np.asarray(angles)
    orig_shape = x.shape
    xf = np.ascontiguousarray(x.reshape(R_TOTAL, DIM), dtype=np.float32)
    ang = gather_angles(angles.astype(np.float32, copy=False))

    nc = _get_nc()
    in_maps = [
        {"x": np.ascontiguousarray(xf[c * R_CORE:(c + 1) * R_CORE]),
         "ang": ang, "consts": _CONSTS}
        for c in range(N_CORES)
    ]
    trace = os.environ.get("BFK_TRACE", "") == "1"
    res = run_bass_kernel_spmd(nc, in_maps, list(range(N_CORES)), trace=trace)
    LAST_RESULT = res
    out = np.concatenate(
        [np.asarray(res.results[c]["out"]).astype(np.float32)
         for c in range(N_CORES)], axis=0)
    return out.reshape(orig_shape).astype(x.dtype, copy=False)


# revision 4
# speedup vs baseline: 2.6988x; 1.1103x over previous
"""Blockwise butterfly rotation - TRN2 Bass kernel, v5.

out = x @ blockdiag(C_b); C built on device from angles; PE does 32 bf16
transposes + 32 bf16 matmuls per 128-row tile (measured ~6.1us/tile incl.
PSUM drains); DMA: f32 in (HWDGE sync), bf16 out (SWDGE, no cast).

vs the f32 baseline:
  - bf16 everywhere internally (x downconverted on arrival, bf16 C, bf16
    DRAM output upcast on host), roughly halving PE streaming and all
    PSUM->SBUF drain costs.
  - C build slimmed: butterfly stages + combine in bf16, hss copies on ACT
    (idle during the build), combine merged to one 512-wide mul per block;
    xT buffered 8 deep so the transpose stream hides the build window.
  - Group-granular software pipeline: each macro-cycle emits tile r+3's
    input DMA, tile r+2's bf16 conversion, and alternates tile r+1's
    transpose groups with tile r's matmul groups. MM group g depends only
    on T group g of its own tile (finished a full cycle earlier), so the
    strictly in-order PE never waits on a just-issued PSUM drain, and each
    engine's queue keeps drains adjacent to their producers.
"""
import math
import os

import numpy as np

from concourse import bacc, mybir, tile
from concourse.bass_utils import run_bass_kernel_spmd

F32 = mybir.dt.float32
BF16 = mybir.dt.bfloat16

DIM = 4096
NB = 16
BLOCK = 256
HALF_PI = math.pi / 2.0

N_CORES = 8
R_TOTAL = 4 * 4096
R_CORE = R_TOTAL // N_CORES  # 2048

_C_ID = 0
_C_PI = 128
_C_LS = 129
_C_HSB = 641
_C_W = 1153
_C_COLS = 3201

LAST_RESULT = None
_NC_CACHE = {}

BUILD_DT = mybir.dt.bfloat16 if os.environ.get("BFK_BUILD", "bf16") == "bf16" \
    else mybir.dt.float32


def _build_consts() -> np.ndarray:
    c = np.zeros((128, _C_COLS), dtype=np.float32)
    p = np.arange(128)
    c[:, _C_ID:_C_ID + 128] = np.eye(128, dtype=np.float32)
    c[:, _C_PI] = HALF_PI
    ls = np.zeros((128, 16, 2, 16), np.float32)
    ls[p, :, :, p % 16] = 1.0
    c[:, _C_LS:_C_LS + 512] = ls.reshape(128, 512)
    hsb = np.zeros((128, 2, 16, 16), np.float32)
    for kc in range(2):
        hsb[:, kc, :, :] = (np.arange(16)[None, :] == (8 * kc + p // 16)[:, None])[:, None, :]
    c[:, _C_HSB:_C_HSB + 512] = hsb.reshape(128, 512)
    w = np.zeros((128, 16, 8, 16), np.float32)
    for b in range(16):
        for mg in range(8):
            w[16 * mg + b, b, mg, :] = 1.0
    c[:, _C_W:_C_W + 2048] = w.reshape(128, 2048)
    return c


_CONSTS = _build_consts()


def gather_angles(angles: np.ndarray) -> np.ndarray:
    """angles [16, 8, 128] f32 -> ang [128, 1536] f32 (angL 4x256 | angH 4x128)."""
    angles = np.asarray(angles)
    assert angles.shape == (NB, 8, 128)
    out = np.empty((128, 1536), dtype=np.float32)
    for s in range(4):
        sig = 1 << s
        col = np.empty((128, 256), dtype=np.float32)
        for g0 in range(8):
            row = np.empty((16, 2, 8), dtype=np.float32)
            for kc in range(2):
                g = 8 * kc + g0
                for vg in range(8 // sig):
                    for t in range(sig):
                        row[:, kc, vg * sig + t] = angles[:, s, 8 * g + vg * sig + t]
            col[16 * g0:16 * g0 + 16, :] = row.reshape(1, 256)
        out[:, 256 * s:256 * (s + 1)] = col
    for sp in range(4):
        sigp = 1 << sp
        col = np.empty((128, 128), dtype=np.float32)
        for b in range(16):
            row = np.empty((16, 8), dtype=np.float32)
            for v in range(16):
                for wg in range(8 // sigp):
                    for t in range(sigp):
                        row[v, wg * sigp + t] = angles[b, sp + 4, wg * 16 * sigp + 16 * t + v]
            col[b::16, :] = row.reshape(1, 128)
        out[:, 1024 + 128 * sp:1024 + 128 * (sp + 1)] = col
    return out


def _butterfly_stage(nc, pool, data, n1, n2, sig, cos_ap, sin_ap, dt):
    ng = 8 // sig
    v = data.rearrange("p (n1 n2 vg h t) -> p n1 n2 vg h t",
                       n1=n1, n2=n2, vg=ng, h=2, t=sig)
    a = v[:, :, :, :, 0, :]
    b_ = v[:, :, :, :, 1, :]
    half = n1 * n2 * 8
    t1 = pool.tile([128, half], dt, name="bt_t1", tag="bt_t1")
    t2 = pool.tile([128, half], dt, name="bt_t2", tag="bt_t2")
    t3 = pool.tile([128, half], dt, name="bt_t3", tag="bt_t3")
    t4 = pool.tile([128, half], dt, name="bt_t4", tag="bt_t4")
    tv = lambda t: t[:].rearrange("p (n1 n2 vg t) -> p n1 n2 vg t",
                                  n1=n1, n2=n2, vg=ng, t=sig)
    nc.vector.tensor_mul(tv(t1), a, cos_ap)
    nc.vector.tensor_mul(tv(t2), b_, sin_ap)
    nc.vector.tensor_mul(tv(t3), a, sin_ap)
    nc.vector.tensor_mul(tv(t4), b_, cos_ap)
    nc.vector.tensor_sub(a, tv(t1), tv(t2))
    nc.vector.tensor_add(b_, tv(t3), tv(t4))


def build_nc(R: int, repeat: int | None = None, repeat_scope: str = "main"):
    import contextlib
    assert R % 128 == 0
    RT = R // 128
    nc = bacc.Bacc("TRN2", target_bir_lowering=False, debug=False)
    BDT = BUILD_DT

    X = nc.dram_tensor("x", [R, DIM], F32, kind="ExternalInput").ap()
    ANG = nc.dram_tensor("ang", [128, 1536], F32, kind="ExternalInput").ap()
    CIN = nc.dram_tensor("consts", [128, _C_COLS], F32, kind="ExternalInput").ap()
    OUT = nc.dram_tensor("out", [R, DIM], BF16, kind="ExternalOutput").ap()

    with tile.TileContext(nc) as tc:
        with tc.tile_pool(name="const", bufs=1) as cpool, \
             tc.tile_pool(name="build", bufs=1) as bpool, \
             tc.tile_pool(name="xin", bufs=3) as xpool, \
             tc.tile_pool(name="xinb", bufs=3) as xbpool, \
             tc.tile_pool(name="xt", bufs=8) as xtpool, \
             tc.tile_pool(name="outp", bufs=2) as opool, \
             tc.tile_pool(name="psR", bufs=2, space="PSUM") as psR, \
             tc.tile_pool(name="psT", bufs=3, space="PSUM") as psT, \
             tc.tile_pool(name="psO", bufs=3, space="PSUM") as psO:
            consts = cpool.tile([128, _C_COLS], F32)
            nc.sync.dma_start(out=consts[:], in_=CIN)
            ident = consts[:, _C_ID:_C_ID + 128]
            halfpi = consts[:, _C_PI:_C_PI + 1]
            identb = cpool.tile([128, 128], BF16)
            nc.vector.tensor_copy(identb[:], ident)
            CT = cpool.tile([128, 8192], BF16)  # C: [p=k%128, (b, kc, w, v)]

            outer_cm = (tc.For_i(0, repeat, 1)
                        if repeat and repeat_scope == "all"
                        else contextlib.nullcontext())
            outer_cm.__enter__()

            # ---------------- C build ----------------
            angsb = bpool.tile([128, 1536], F32)
            nc.sync.dma_start(out=angsb[:], in_=ANG)

            LS = bpool.tile([128, 512], BDT)
            nc.vector.tensor_copy(LS[:], consts[:, _C_LS:_C_LS + 512])
            HSB = bpool.tile([128, 512], BDT)
            nc.vector.tensor_copy(HSB[:], consts[:, _C_HSB:_C_HSB + 512])

            # HSB stages 4-7 first (gate the selector matmuls).
            for sp in range(4):
                sigp = 1 << sp
                ng = 8 // sigp
                cosT = bpool.tile([128, 128], BDT, name="cosH", tag="cosH")
                sinT = bpool.tile([128, 128], BDT, name="sinH", tag="sinH")
                asl = angsb[:, 1024 + 128 * sp:1024 + 128 * (sp + 1)]
                nc.scalar.activation(cosT[:], asl, mybir.ActivationFunctionType.Sin,
                                     bias=halfpi, scale=1.0)
                nc.scalar.activation(sinT[:], asl, mybir.ActivationFunctionType.Sin,
                                     bias=0.0, scale=1.0)
                cv = cosT[:].rearrange("p (v vg t) -> p v vg t", v=16, vg=ng, t=sigp) \
                    .unsqueeze(1).to_broadcast((128, 2, 16, ng, sigp))
                sv = sinT[:].rearrange("p (v vg t) -> p v vg t", v=16, vg=ng, t=sigp) \
                    .unsqueeze(1).to_broadcast((128, 2, 16, ng, sigp))
                _butterfly_stage(nc, bpool, HSB[:], 2, 16, sigp, cv, sv, BDT)

            # LS stages 0-3
            for s in range(4):
                sig = 1 << s
                ng = 8 // sig
                cosT = bpool.tile([128, 256], BDT, name="cosL", tag="cosL")
                sinT = bpool.tile([128, 256], BDT, name="sinL", tag="sinL")
                asl = angsb[:, 256 * s:256 * (s + 1)]
                nc.scalar.activation(cosT[:], asl, mybir.ActivationFunctionType.Sin,
                                     bias=halfpi, scale=1.0)
                nc.scalar.activation(sinT[:], asl, mybir.ActivationFunctionType.Sin,
                                     bias=0.0, scale=1.0)
                cv = cosT[:].rearrange("p (b kc vg t) -> p b kc vg t",
                                       b=16, kc=2, vg=ng, t=sig)
                sv = sinT[:].rearrange("p (b kc vg t) -> p b kc vg t",
                                       b=16, kc=2, vg=ng, t=sig)
                _butterfly_stage(nc, bpool, LS[:], 16, 2, sig, cv, sv, BDT)

            # Selector matmuls need HSB in a PE dtype. HSB is bf16 (or f32
            # fallback -> use f32r for full-rate); Wb is f32 -> make a bf16 W.
            Wball = cpool.tile([128, 2048], BF16)
            nc.scalar.copy(Wball[:], consts[:, _C_W:_C_W + 2048])
            HSBmm = HSB
            if BDT != BF16:
                HSBmm = bpool.tile([128, 512], BF16)
                nc.vector.tensor_copy(HSBmm[:], HSB[:])

            for b in range(16):
                Wb = Wball[:, 128 * b:128 * (b + 1)]
                psr = psR.tile([128, 512], F32, name="psr", tag="psr")
                nc.tensor.matmul(psr[:], Wb, HSBmm[:], start=True, stop=True)
                hss = bpool.tile([128, 512], BDT, name="hss", tag="hss", bufs=2)
                nc.scalar.copy(hss[:], psr[:])
                # CT[p, b, kc, w, v] = LS[p, b, kc, v] * hss[p, kc, v, w],
                # one 512-wide mul per block.
                o = CT[:, b * 512:(b + 1) * 512] \
                    .rearrange("p (kc w v) -> p kc w v", kc=2, w=16, v=16)
                i0 = LS[:, b * 32:(b + 1) * 32] \
                    .rearrange("p (kc v) -> p kc v", kc=2, v=16) \
                    .unsqueeze(2).to_broadcast((128, 2, 16, 16))
                i1 = hss[:].rearrange("p (kc v w) -> p kc w v", kc=2, v=16, w=16)
                nc.vector.tensor_mul(o, i0, i1)

            # ---------------- main loop ----------------
            inner_cm = (tc.For_i(0, repeat, 1)
                        if repeat and repeat_scope == "main"
                        else contextlib.nullcontext())
            def emit_dma(r):
                xin32 = xpool.tile([128, DIM], F32, name="xin32", tag="xin32")
                nc.sync.dma_start(out=xin32[:], in_=X[r * 128:(r + 1) * 128, :])
                return xin32

            def emit_conv(xin32):
                xin = xbpool.tile([128, DIM], BF16, name="xin", tag="xin")
                nc.vector.tensor_copy(xin[:, 0:3072], xin32[:, 0:3072])
                nc.scalar.copy(xin[:, 3072:4096], xin32[:, 3072:4096])
                return xin

            def emit_tgroup(xin, xT, j):
                pst = psT.tile([128, 512], BF16, name="pst", tag="pst")
                for q in range(4):
                    i = 4 * j + q
                    nc.tensor.transpose(
                        pst[:, 128 * q:128 * (q + 1)],
                        xin[:, 128 * i:128 * (i + 1)], identb[:])
                if j % 2 == 0:
                    nc.vector.tensor_copy(xT[:, 512 * j:512 * (j + 1)], pst[:])
                else:
                    nc.scalar.copy(xT[:, 512 * j:512 * (j + 1)], pst[:])

            def emit_mmgroup(xT, outt, jb):
                pso = psO.tile([128, 512], F32, name="pso", tag="pso")
                for q in range(2):
                    b = 2 * jb + q
                    for kc in range(2):
                        i = 2 * b + kc
                        nc.tensor.matmul(
                            pso[:, 256 * q:256 * (q + 1)],
                            xT[:, 128 * i:128 * (i + 1)],
                            CT[:, 256 * i:256 * (i + 1)],
                            start=(kc == 0), stop=(kc == 1))
                if jb in (1, 5):
                    nc.scalar.copy(outt[:, 512 * jb:512 * (jb + 1)], pso[:])
                else:
                    nc.vector.tensor_copy(outt[:, 512 * jb:512 * (jb + 1)], pso[:])

            # Group-granular software pipeline with a 2-tile conversion lead:
            # each macro-cycle r emits tile r+2's load/convert, then alternates
            # tile r+1's transpose groups with tile r's matmul groups. MM
            # group g only needs T group g of its own tile (finished a full
            # cycle earlier), so the strictly in-order PE never waits on a
            # just-issued PSUM drain; DVE/ACT queues keep drains adjacent to
            # their producers.
            with inner_cm:
                x32s, xins, xTs = {}, {}, {}
                for k in range(min(3, RT)):
                    x32s[k] = emit_dma(k)
                for k in range(min(2, RT)):
                    xins[k] = emit_conv(x32s.pop(k))
                xTs[0] = xtpool.tile([128, DIM], BF16, name="xT", tag="xT")
                for j in range(8):
                    emit_tgroup(xins[0], xTs[0], j)
                for r in range(RT):
                    if r + 3 < RT:
                        x32s[r + 3] = emit_dma(r + 3)
                    if r + 2 < RT:
                        xins[r + 2] = emit_conv(x32s.pop(r + 2))
                    if r + 1 < RT:
                        xTs[r + 1] = xtpool.tile([128, DIM], BF16,
                                                 name="xT", tag="xT")
                    outt = opool.tile([128, DIM], BF16, name="outt", tag="outt")
                    for g in range(8):
                        if r + 1 < RT:
                            emit_tgroup(xins[r + 1], xTs[r + 1], g)
                        emit_mmgroup(xTs[r], outt, g)
                    nc.gpsimd.dma_start(out=OUT[r * 128:(r + 1) * 128, :],
                                        in_=outt[:])
                    xins.pop(r, None)
                    xTs.pop(r, None)
            outer_cm.__exit__(None, None, None)

    nc.compile()
    return nc


def _get_nc():
    if "nc" not in _NC_CACHE:
        _NC_CACHE["nc"] = build_nc(R_CORE)
    return _NC_CACHE["nc"]


def kernel(x: np.ndarray, angles: np.ndarray) -> np.ndarray:
    global LAST_RESULT
    x = np.asarray(x)
    angles = np.asarray(angles)
    orig_shape = x.shape
    xf = np.ascontiguousarray(x.reshape(R_TOTAL, DIM), dtype=np.float32)
    ang = gather_angles(angles.astype(np.float32, copy=False))

    nc = _get_nc()
    in_maps = [
        {"x": np.ascontiguousarray(xf[c * R_CORE:(c + 1) * R_CORE]),
         "ang": ang, "consts": _CONSTS}
        for c in range(N_CORES)
    ]
    trace = os.environ.get("BFK_TRACE", "") == "1"
    res = run_bass_kernel_spmd(nc, in_maps, list(range(N_CORES)), trace=trace)
    LAST_RESULT = res
    out = np.concatenate(
        [np.asarray(res.results[c]["out"]).astype(np.float32)
         for c in range(N_CORES)], axis=0)
    return out.reshape(orig_shape).astype(x.dtype, copy=False)
